# revision 48
# baseline (speedup 1.0000x reference)
"""Trainium2 Bass kernel for nn_MultiHeadAttention (B=4, S=2048, C=256, H=8).

Sharding: data-parallel over (batch, seq) — 8 cores, core i handles
batch b = i//2 and query rows r0 = (i%2)*1024 .. r0+1024.  Each core
computes K/V projections for its full batch sequence (all 8 heads),
attention + fc for its 1024 query rows, then residual + LayerNorm.
No collectives needed; host concatenates the 8 row-shards.

fp8 (e4m3) DoubleRow matmuls everywhere (2x PE throughput).  The final
output is dominated by the residual (attention contribution has sigma
~0.036 vs residual sigma 1), so fp8 noise in the attention path is
suppressed ~30x in the output — rel err lands ~8e-3 vs the 2e-2 gate.

Scores are re-associated: s = x @ G with G = (2Wk^T) @ (16(Wq x^T+bq)),
so there is NO K projection, the scores lhsT is the static xt8, and bk
drops out exactly (it only adds a per-query-row constant, which
softmax cancels).  Scaling scheme so every fp8 tensor has sigma ~O(1):
  Wq x16, Wk^T x2 host-side -> qt sigma 16, G sigma 32; exp scale 1/512
  exp bias -3               -> e below the e4m3 max (240)
  rowsum ones = 0.125       -> ot = 8 * (softmax @ v), sigma ~0.29
  Wfc x32 host-side         -> fc psum = 256 * fc_out
  residual+bias x256 host-side, LN eps x256^2: LN is scale-invariant
PSUM drains alternate ACT/DVE so consecutive PSUM slots free in
parallel; the PE stream is software-pipelined one pair ahead and
across chunk boundaries so it never waits on the ACT exp.  fc
accumulates over all 8 heads in PSUM at the tail (per-head ot tiles
stay resident in SBUF fp8); chunk-0's fc+LayerNorm overlap the last
head's chunk-1 attention.

Every DMA writes a persistent SBUF buffer or a freshly-allocated pool
tile so each DMA instruction needs at most one semaphore wait.
"""

import sys

for _p in ("/opt/trn_rl_repo",):
    if _p not in sys.path:
        sys.path.insert(0, _p)

from contextlib import ExitStack

import numpy as np

import concourse.bass as bass
from concourse import bacc
import concourse.tile as tile
from concourse import mybir
from concourse.masks import make_identity

P = 128
B, S, C, H = 4, 2048, 256, 8
RQ = 1024            # query rows per core
CH = 512             # query-row chunk (matmul N)
NCH = RQ // CH       # chunks per core = 2
NT = S // P          # t tiles = 16
ND = C // P          # d tiles = 2
NR = RQ // P         # row tiles per core = 8
EPS = 1e-5

WQK_SCALE = 16.0     # host premultiplier on Wq, bq
WKT_SCALE = 2.0      # host premultiplier on Wk^T (G = 2Wk^T @ qt, sigma~32)
WV_SCALE = 1.0       # host premultiplier on Wv
WFC_SCALE = 32.0     # host premultiplier on Wfc
RES_SCALE = 256.0    # host premultiplier on residual + bfc_eff
# scores s = x @ G = 32*16 * K@(Q+bq); true score = K@(Q+bq)/sqrt(C)
SCALE_EXP = 1.0 / (np.sqrt(C) * WQK_SCALE * WKT_SCALE)  # = 1/512
BIAS_EXP = -3.0      # keeps exp() below e4m3 max (240); cancels in softmax
ONES_VAL = 0.125     # rowsum lhsT value -> ot = 8 * head_out
EPS_EFF = float(EPS * RES_SCALE * RES_SCALE)

F32 = mybir.dt.float32
BF16 = mybir.dt.bfloat16
FP8 = mybir.dt.float8e4
AF = mybir.ActivationFunctionType
OP = mybir.AluOpType
DR = mybir.MatmulPerfMode.DoubleRow


def build_nc() -> bass.Bass:
    nc = bacc.Bacc(None)

    xt8 = nc.declare_dram_parameter("xt8", [P, ND, S], FP8, isOutput=False)
    xb8 = nc.declare_dram_parameter("xb8", [P, NT, C], FP8, isOutput=False)
    xqf = nc.declare_dram_parameter("xqf", [P, NR, C], F32, isOutput=False)
    wq8 = nc.declare_dram_parameter("wq8", [P, ND, H, C], FP8, isOutput=False)
    # wkT8 = 2 * Wk^T packed [d_ci, d2, h, c]; bk is dropped entirely (it
    # only adds a per-query-row constant to scores, which softmax cancels)
    wkT8 = nc.declare_dram_parameter("wkT8", [P, ND, H, C], FP8, isOutput=False)
    wv8 = nc.declare_dram_parameter("wv8", [P, ND, H, C], FP8, isOutput=False)
    wfc8 = nc.declare_dram_parameter("wfc8", [P, ND, H, C], FP8, isOutput=False)
    # bq8 = host-packed [P, ND, H] = 16*bq
    bqk = nc.declare_dram_parameter("bqk", [P, ND, H], F32, isOutput=False)
    gb = nc.declare_dram_parameter("gb", [2, C], F32, isOutput=False)
    out = nc.declare_dram_parameter("out", [RQ, C], F32, isOutput=True)

    with tile.TileContext(nc) as tc, ExitStack() as ctx:
        singles = ctx.enter_context(tc.tile_pool(name="singles", bufs=1))
        hpool = ctx.enter_context(tc.tile_pool(name="hpool", bufs=2))
        epool = ctx.enter_context(tc.tile_pool(name="epool", bufs=2))
        opool = ctx.enter_context(tc.tile_pool(name="opool", bufs=2))
        onpool = ctx.enter_context(tc.tile_pool(name="onpool", bufs=2))
        lnpool = ctx.enter_context(tc.tile_pool(name="lnpool", bufs=4))

        # PSUM: pst 2x[128,1024] = 4 banks, pot 3x[128,512] = 3, prs 1 = 1
        pst = ctx.enter_context(tc.tile_pool(name="pst", bufs=2, space="PSUM"))
        pot = ctx.enter_context(tc.tile_pool(name="pot", bufs=3, space="PSUM"))
        prs = ctx.enter_context(tc.tile_pool(name="prs", bufs=1, space="PSUM"))

        # ---- constants ----
        ident = singles.tile([P, P], BF16)
        make_identity(nc, ident)
        ones2 = singles.tile([P, ND, P], FP8)
        nc.vector.memset(ones2, ONES_VAL)
        eps_t = singles.tile([P, 1], F32)
        nc.vector.memset(eps_t, EPS_EFF)
        nb2_t = singles.tile([P, 1], F32)
        nc.vector.memset(nb2_t, BIAS_EXP)

        # ---- input DMAs (host pre-packs exact SBUF layouts) ----
        wv_sb = singles.tile([P, ND, H, C], FP8)
        wk_sb = singles.tile([P, ND, H, C], FP8)
        wq_sb = singles.tile([P, ND, H, C], FP8)
        wfc_sb = singles.tile([P, ND, H, C], FP8)
        # head 0+1 weight slices land first so compute starts ~1.5us in
        for hh in ((0, 2), (2, 8)):
            for w_sb, dram in ((wq_sb, wq8), (wk_sb, wkT8), (wv_sb, wv8)):
                nc.sync.dma_start(out=w_sb[:, :, hh[0]:hh[1]],
                                  in_=dram[:, :, hh[0]:hh[1]])
        nc.sync.dma_start(out=wfc_sb, in_=wfc8[:])
        xt_sb = singles.tile([P, ND, S], FP8)
        for q4 in range(4):
            nc.scalar.dma_start(out=xt_sb[:, :, q4 * 512:(q4 + 1) * 512],
                                in_=xt8[:, :, q4 * 512:(q4 + 1) * 512])
        xb_sb = singles.tile([P, NT, C], FP8)
        for q4 in range(2):
            nc.scalar.dma_start(out=xb_sb[:, q4 * 8:(q4 + 1) * 8, :],
                                in_=xb8[:, q4 * 8:(q4 + 1) * 8, :])
        bqk_sb = singles.tile([P, ND, H], F32)
        nc.scalar.dma_start(out=bqk_sb, in_=bqk[:])
        xqf_sb = singles.tile([P, NR, C], F32)
        nc.scalar.dma_start(out=xqf_sb, in_=xqf[:])
        gb_sb = singles.tile([P, 2, C], F32)
        gb_ap = gb[:]
        gb_bc = bass.AP(tensor=gb_ap.tensor, offset=gb_ap.offset,
                        ap=[[0, P]] + list(gb_ap.ap))
        nc.scalar.dma_start(out=gb_sb, in_=gb_bc)
        gamma_sb = gb_sb[:, 0]
        beta_sb = gb_sb[:, 1]

        # ---- per-head ot output staging (fp8, all heads resident) ----
        ot_all = singles.tile([P, H, NCH, ND, CH], FP8)

        # ---- PE warmup: dummy matmuls while input DMAs land (HAM clock).
        # 12 up front cover the first weight/x DMAs; the rest are emitted
        # between head-0's projection groups so their PSUM drains overlap
        # warmup instead of stalling the first scores.
        warm = prs.tile([P, CH], F32, tag="rs", name="warm")
        for _ in range(12):
            nc.tensor.matmul(warm[:, 0:P], lhsT=ident, rhs=ident,
                             start=True, stop=True)

        out_r = out.rearrange("(n p) d -> p n d", p=P)

        # tail for one chunk: fc over all heads (PSUM-accumulated) + LN
        def emit_tail(ch):
            for r1 in range(CH // P):
                idx = ch * (CH // P) + r1
                fcp = pot.tile([P, CH], F32, tag="ot", name="fcp")
                for hh in range(H):
                    nc.tensor.matmul(
                        fcp[:, 0:C],
                        lhsT=ot_all[:, hh, ch, :, r1 * P:(r1 + 1) * P],
                        rhs=wfc_sb[:, :, hh, :],
                        start=(hh == 0), stop=(hh == H - 1), perf_mode=DR,
                    )
                u = lnpool.tile([P, C], F32, tag="u", bufs=3)
                # u = fc_psum + 256*(resid + bfc_eff); LN is scale-invariant
                nc.vector.tensor_tensor(out=u, in0=fcp[:, 0:C],
                                        in1=xqf_sb[:, idx], op=OP.add)
                stats = lnpool.tile([P, 6], F32, tag="stats")
                nc.vector.bn_stats(out=stats, in_=u)
                mv = lnpool.tile([P, 2], F32, tag="mv")
                nc.vector.bn_aggr(out=mv, in_=stats)
                sd = lnpool.tile([P, 1], F32, tag="sd")
                nc.scalar.activation(out=sd, in_=mv[:, 1:2], func=AF.Sqrt,
                                     bias=eps_t, scale=1.0)
                rstd = lnpool.tile([P, 1], F32, tag="rstd")
                nc.vector.reciprocal(out=rstd, in_=sd)
                # (u - mu)*gamma on DVE, *rstd on ACT, +beta on GpSimd
                nc.vector.scalar_tensor_tensor(
                    out=u, in0=u, scalar=mv[:, 0:1], in1=gamma_sb,
                    op0=OP.subtract, op1=OP.mult)
                nc.scalar.activation(out=u, in_=u, func=AF.Copy, scale=rstd)
                nc.gpsimd.tensor_tensor(out=u, in0=u, in1=beta_sb, op=OP.add)
                nc.gpsimd.dma_start(out=out_r[:, idx:idx + 1, :], in_=u)

        # ---- head loop ----
        # Projections: qt = 16*(Wq x^T + bq), then G = 2Wk^T @ qt, so the
        # scores matmul is x @ G with a STATIC lhsT (xt8) — no K projection
        # at all.  Head h+1's projection groups are interleaved into head
        # h's chunk streams (one PSUM group per odd tp) and the first scores
        # of head h+1 are prefetched before head h's last AV, so the PE
        # stream is seamless across heads.  Drains alternate ACT/DVE so
        # consecutive PSUM slots free in parallel.
        def make_head_ctx(h):
            qt = hpool.tile([P, ND, RQ], FP8, tag="qt", name="qt")
            gt = hpool.tile([P, ND, RQ], FP8, tag="gt", name="gt")
            v_sb = hpool.tile([P, NT, C], FP8, tag="v", name="v_sb")
            e_tiles = [epool.tile([P, NT, CH], FP8, tag="e", name=f"e{c}")
                       for c in range(NCH)]

            def emit_q(d2, eng):
                qps = pst.tile([P, 1024], F32, tag="st", name="qps")
                for i in range(2):
                    nc.tensor.matmul(
                        qps[:, i * 512:(i + 1) * 512],
                        lhsT=wq_sb[:, :, h, d2 * P:(d2 + 1) * P],
                        rhs=xt_sb[:, :, i * 512:(i + 1) * 512],
                        start=True, stop=True, perf_mode=DR,
                    )
                if eng == "act":
                    nc.scalar.activation(
                        out=qt[:, d2, :], in_=qps, func=AF.Identity,
                        bias=bqk_sb[:, d2, h:h + 1], scale=1.0)
                else:
                    nc.vector.tensor_scalar_add(
                        out=qt[:, d2, :], in0=qps,
                        scalar1=bqk_sb[:, d2, h:h + 1])

            def emit_g(c2, eng):
                gps = pst.tile([P, 1024], F32, tag="st", name="gps")
                for i in range(2):
                    nc.tensor.matmul(
                        gps[:, i * 512:(i + 1) * 512],
                        lhsT=wk_sb[:, :, h, c2 * P:(c2 + 1) * P],
                        rhs=qt[:, :, i * 512:(i + 1) * 512],
                        start=True, stop=True, perf_mode=DR,
                    )
                if eng == "act":
                    nc.scalar.activation(out=gt[:, c2, :], in_=gps,
                                         func=AF.Copy)
                else:
                    nc.vector.tensor_copy(out=gt[:, c2, :], in_=gps)

            def emit_v(q4, eng):
                vps = pst.tile([P, 1024], F32, tag="st", name="vps")
                for i in range(4):
                    t = q4 * 4 + i
                    nc.tensor.matmul(
                        vps[:, i * C:(i + 1) * C],
                        lhsT=xt_sb[:, :, t * P:(t + 1) * P],
                        rhs=wv_sb[:, :, h, :],
                        start=True, stop=True, perf_mode=DR,
                    )
                if eng == "act":
                    nc.scalar.activation(out=v_sb[:, q4 * 4:(q4 + 1) * 4, :],
                                         in_=vps, func=AF.Copy)
                else:
                    nc.vector.tensor_copy(out=v_sb[:, q4 * 4:(q4 + 1) * 4, :],
                                          in_=vps)

            thunks = [
                lambda: emit_q(0, "act"),
                lambda: emit_q(1, "dve"),
                lambda: emit_g(0, "dve"),
                lambda: emit_g(1, "act"),
                lambda: emit_v(0, "act"),
                lambda: emit_v(1, "dve"),
                lambda: emit_v(2, "act"),
                lambda: emit_v(3, "dve"),
            ]
            return dict(qt=qt, gt=gt, v=v_sb, e=e_tiles, thunks=thunks)

        def emit_scores(hc, ch, tp):
            stp = pst.tile([P, 1024], F32, tag="st", name="stp")
            for i in range(2):
                t = tp * 2 + i
                nc.tensor.matmul(
                    stp[:, i * CH:(i + 1) * CH],
                    lhsT=xt_sb[:, :, t * P:(t + 1) * P],
                    rhs=hc["gt"][:, :, ch * CH:(ch + 1) * CH],
                    start=True, stop=True, perf_mode=DR,
                )
            # e = exp(scores/512 - 3), fp8; -3 cancels in softmax
            nc.scalar.activation(out=hc["e"][ch][:, 2 * tp:2 * tp + 2, :],
                                 in_=stp, func=AF.Exp,
                                 scale=float(SCALE_EXP), bias=nb2_t)

        cur = make_head_ctx(0)
        for t_ in cur["thunks"]:
            t_()
            for _ in range(4):
                nc.tensor.matmul(warm[:, 0:P], lhsT=ident, rhs=ident,
                                 start=True, stop=True)
        emit_scores(cur, 0, 0)
        for h in range(H):
            nxt = make_head_ctx(h + 1) if h < H - 1 else None
            pend = list(nxt["thunks"]) if nxt else []
            for ch in range(NCH):
                e_sb = cur["e"][ch]
                v_sb = cur["v"]
                otp = [pot.tile([P, CH], F32, tag="ot", name=f"ot{d2}")
                       for d2 in range(ND)]
                rs = prs.tile([P, CH], F32, tag="rs", name="rs")

                for tp in range(NT // 2):
                    if tp < NT // 2 - 1:
                        emit_scores(cur, ch, tp + 1)
                    elif ch == 0:
                        emit_scores(cur, 1, 0)
                    elif nxt is not None:
                        emit_scores(nxt, 0, 0)  # cross-head prefetch
                    nc.tensor.matmul(rs, lhsT=ones2,
                                     rhs=e_sb[:, 2 * tp:2 * tp + 2, :],
                                     start=(tp == 0), stop=(tp == NT // 2 - 1),
                                     perf_mode=DR)
                    for d2 in range(ND):
                        nc.tensor.matmul(
                            otp[d2],
                            lhsT=v_sb[:, 2 * tp:2 * tp + 2, d2 * P:(d2 + 1) * P],
                            rhs=e_sb[:, 2 * tp:2 * tp + 2, :],
                            start=(tp == 0), stop=(tp == NT // 2 - 1),
                            perf_mode=DR,
                        )
                    if tp % 2 == 1 and pend:
                        pend.pop(0)()

                rcp = opool.tile([P, CH], F32, tag="rcp")
                nc.vector.reciprocal_approx_fast(out=rcp, in_=rs)
                for d2 in range(ND):
                    nc.vector.tensor_tensor(
                        out=ot_all[:, h, ch, d2, :], in0=otp[d2], in1=rcp,
                        op=OP.mult)
                # chunk-0 tail overlaps the last head's chunk-1 attention
                if h == H - 1:
                    emit_tail(ch)
            cur = nxt

    nc.finalize()
    return nc


_NC = None


def _get_nc():
    global _NC
    if _NC is None:
        _NC = build_nc()
    return _NC


def make_in_maps(inputs):
    import ml_dtypes
    f8 = ml_dtypes.float8_e4m3
    x = np.asarray(inputs["x"], dtype=np.float32)
    wq = np.asarray(inputs["Wq"], np.float32) * WQK_SCALE
    wk = np.asarray(inputs["Wk"], np.float32) * WKT_SCALE
    wv = np.asarray(inputs["Wv"], np.float32) * WV_SCALE
    wfc = np.asarray(inputs["Wfc"], np.float32)
    bfc_eff = (np.asarray(inputs["bfc"], np.float32).ravel()
               + np.asarray(inputs["bv"], np.float32).ravel() @ wfc)

    def pack_w(w):  # [H, C, C] -> [P, ND, H, C]
        return np.ascontiguousarray(
            w.reshape(H, ND, P, C).transpose(2, 1, 0, 3).astype(f8))

    shared = {
        "wq8": pack_w(wq),
        "wkT8": pack_w(np.ascontiguousarray(wk.transpose(0, 2, 1))),
        "wv8": pack_w(wv),
        "wfc8": pack_w((wfc * WFC_SCALE).reshape(H, C, C)),
        "bqk": np.ascontiguousarray(
            (np.asarray(inputs["bq"], np.float32) * WQK_SCALE)
            .reshape(H, ND, P).transpose(2, 1, 0)),
        "gb": np.ascontiguousarray(np.stack([
            np.asarray(inputs["gamma"], np.float32).ravel(),
            np.asarray(inputs["beta"], np.float32).ravel(),
        ])),
    }
    in_maps = []
    for core in range(8):
        b, r0 = core // 2, (core % 2) * RQ
        m = dict(shared)
        xr = np.roll(x[b], -r0, axis=0)  # query rows first
        m["xt8"] = np.ascontiguousarray(
            xr.T.reshape(ND, P, S).transpose(1, 0, 2).astype(f8))
        m["xb8"] = np.ascontiguousarray(
            xr.reshape(NT, P, C).transpose(1, 0, 2).astype(f8))
        m["xqf"] = np.ascontiguousarray(
            ((x[b, r0:r0 + RQ] + bfc_eff[None, :]) * RES_SCALE)
            .reshape(NR, P, C).transpose(1, 0, 2))
        in_maps.append(m)
    return in_maps


def assemble(results):
    out = np.empty((B, S, C), dtype=np.float32)
    for core in range(8):
        b, r0 = core // 2, (core % 2) * RQ
        out[b, r0:r0 + RQ] = results[core]["out"].reshape(RQ, C)
    return out


def kernel(**inputs) -> np.ndarray:
    from concourse.bass_utils import run_bass_kernel_spmd

    nc = _get_nc()
    in_maps = make_in_maps(inputs)
    res = run_bass_kernel_spmd(nc, in_maps, core_ids=list(range(8)))
    return assemble(res.results)


# revision 53
# speedup vs baseline: 1.1803x; 1.1803x over previous
"""Trainium2 Bass kernel for nn_MultiHeadAttention (B=4, S=2048, C=256, H=8).

Sharding: data-parallel over (batch, seq) — 8 cores, core i handles
batch b = i//2 and query rows r0 = (i%2)*1024 .. r0+1024.  Each core
computes K/V projections for its full batch sequence (all 8 heads),
attention + fc for its 1024 query rows, then residual + LayerNorm.
No collectives needed; host concatenates the 8 row-shards.

fp8 (e4m3) DoubleRow matmuls everywhere (2x PE throughput).  The final
output is dominated by the residual (attention contribution has sigma
~0.036 vs residual sigma 1), so fp8 noise in the attention path is
suppressed ~30x in the output — rel err lands ~8e-3 vs the 2e-2 gate.

Scores are re-associated: s = x @ G with G = (2Wk^T) @ (16(Wq x^T+bq)),
so there is NO K projection, the scores lhsT is the static xt8, and bk
drops out exactly (it only adds a per-query-row constant, which
softmax cancels).  Scaling scheme so every fp8 tensor has sigma ~O(1):
  Wq x16, Wk^T x2 host-side -> qt sigma 16, G sigma 32; exp scale 1/512
  exp bias -3               -> e below the e4m3 max (240)
  rowsum ones = 0.125       -> ot = 8 * (softmax @ v), sigma ~0.29
  Wfc x32 host-side         -> fc psum = 256 * fc_out
  residual+bias x256 host-side, LN eps x256^2: LN is scale-invariant
PSUM drains alternate ACT/DVE so consecutive PSUM slots free in
parallel; the PE stream is software-pipelined one pair ahead and
across chunk boundaries so it never waits on the ACT exp.  fc
accumulates over all 8 heads in PSUM at the tail (per-head ot tiles
stay resident in SBUF fp8); chunk-0's fc+LayerNorm overlap the last
head's chunk-1 attention.

Every DMA writes a persistent SBUF buffer or a freshly-allocated pool
tile so each DMA instruction needs at most one semaphore wait.
"""

import sys

for _p in ("/opt/trn_rl_repo",):
    if _p not in sys.path:
        sys.path.insert(0, _p)

from contextlib import ExitStack

import numpy as np

import concourse.bass as bass
from concourse import bacc
import concourse.tile as tile
from concourse import mybir
from concourse.masks import make_identity

P = 128
B, S, C, H = 4, 2048, 256, 8
RQ = 1024            # query rows per core
CH = 512             # query-row chunk (matmul N)
NCH = RQ // CH       # chunks per core = 2
NT = S // P          # t tiles = 16
ND = C // P          # d tiles = 2
NR = RQ // P         # row tiles per core = 8
EPS = 1e-5

WQK_SCALE = 16.0     # host premultiplier on Wq, bq
WKT_SCALE = 2.0      # host premultiplier on Wk^T (G = 2Wk^T @ qt, sigma~32)
WV_SCALE = 1.0       # host premultiplier on Wv
WFC_SCALE = 32.0     # host premultiplier on Wfc
RES_SCALE = 256.0    # host premultiplier on residual + bfc_eff
# scores s = x @ G = 32*16 * K@(Q+bq); true score = K@(Q+bq)/sqrt(C)
SCALE_EXP = 1.0 / (np.sqrt(C) * WQK_SCALE * WKT_SCALE)  # = 1/512
BIAS_EXP = -3.0      # keeps exp() below e4m3 max (240); cancels in softmax
ONES_VAL = 0.125     # rowsum lhsT value -> ot = 8 * head_out
EPS_EFF = float(EPS * RES_SCALE * RES_SCALE)

F32 = mybir.dt.float32
BF16 = mybir.dt.bfloat16
FP8 = mybir.dt.float8e4
AF = mybir.ActivationFunctionType
OP = mybir.AluOpType
DR = mybir.MatmulPerfMode.DoubleRow


def build_nc() -> bass.Bass:
    nc = bacc.Bacc(None)

    xt8 = nc.declare_dram_parameter("xt8", [P, ND, S], FP8, isOutput=False)
    xqf = nc.declare_dram_parameter("xqf", [P, NR, C], F32, isOutput=False)
    wq8 = nc.declare_dram_parameter("wq8", [P, ND, H, C], FP8, isOutput=False)
    # wkT8 = 2 * Wk^T packed [d_ci, d2, h, c]; bk is dropped entirely (it
    # only adds a per-query-row constant to scores, which softmax cancels)
    wkT8 = nc.declare_dram_parameter("wkT8", [P, ND, H, C], FP8, isOutput=False)
    wv8 = nc.declare_dram_parameter("wv8", [P, ND, H, C], FP8, isOutput=False)
    wfc8 = nc.declare_dram_parameter("wfc8", [P, ND, H, C], FP8, isOutput=False)
    # bq8 = host-packed [P, ND, H] = 16*bq
    bqk = nc.declare_dram_parameter("bqk", [P, ND, H], F32, isOutput=False)
    gb = nc.declare_dram_parameter("gb", [2, C], F32, isOutput=False)
    out = nc.declare_dram_parameter("out", [RQ, C], F32, isOutput=True)

    with tile.TileContext(nc) as tc, ExitStack() as ctx:
        singles = ctx.enter_context(tc.tile_pool(name="singles", bufs=1))
        hpool = ctx.enter_context(tc.tile_pool(name="hpool", bufs=2))
        epool = ctx.enter_context(tc.tile_pool(name="epool", bufs=2))
        opool = ctx.enter_context(tc.tile_pool(name="opool", bufs=2))
        onpool = ctx.enter_context(tc.tile_pool(name="onpool", bufs=2))
        lnpool = ctx.enter_context(tc.tile_pool(name="lnpool", bufs=4))

        # PSUM: pst 2x[128,1024] = 4 banks, pot 3x[128,512] = 3, prs 1 = 1
        pst = ctx.enter_context(tc.tile_pool(name="pst", bufs=2, space="PSUM"))
        pot = ctx.enter_context(tc.tile_pool(name="pot", bufs=3, space="PSUM"))
        prs = ctx.enter_context(tc.tile_pool(name="prs", bufs=1, space="PSUM"))

        # ---- constants ----
        ident = singles.tile([P, P], BF16)
        make_identity(nc, ident)
        ones2 = singles.tile([P, ND, P], FP8)
        nc.vector.memset(ones2, ONES_VAL)
        eps_t = singles.tile([P, 1], F32)
        nc.vector.memset(eps_t, EPS_EFF)
        nb2_t = singles.tile([P, 1], F32)
        nc.vector.memset(nb2_t, BIAS_EXP)

        # ---- input DMAs (host pre-packs exact SBUF layouts) ----
        wv_sb = singles.tile([P, ND, H, C], FP8)
        wk_sb = singles.tile([P, ND, H, C], FP8)
        wq_sb = singles.tile([P, ND, H, C], FP8)
        wfc_sb = singles.tile([P, ND, H, C], FP8)
        # head 0+1 weight slices land first so compute starts ~1.5us in
        for hh in ((0, 2), (2, 8)):
            for w_sb, dram in ((wq_sb, wq8), (wk_sb, wkT8), (wv_sb, wv8)):
                nc.sync.dma_start(out=w_sb[:, :, hh[0]:hh[1]],
                                  in_=dram[:, :, hh[0]:hh[1]])
        nc.sync.dma_start(out=wfc_sb, in_=wfc8[:])
        # x/bias inputs issue from the Pool queue (25ns sequencer cost vs
        # 667ns on ACT) so head-0's Q/G drains aren't stuck behind DMA issue
        xt_sb = singles.tile([P, ND, S], FP8)
        for q4 in range(4):
            nc.gpsimd.dma_start(out=xt_sb[:, :, q4 * 512:(q4 + 1) * 512],
                                in_=xt8[:, :, q4 * 512:(q4 + 1) * 512])
        bqk_sb = singles.tile([P, ND, H], F32)
        nc.gpsimd.dma_start(out=bqk_sb, in_=bqk[:])
        xqf_sb = singles.tile([P, NR, C], F32)
        nc.gpsimd.dma_start(out=xqf_sb, in_=xqf[:])
        gb_sb = singles.tile([P, 2, C], F32)
        gb_ap = gb[:]
        gb_bc = bass.AP(tensor=gb_ap.tensor, offset=gb_ap.offset,
                        ap=[[0, P]] + list(gb_ap.ap))
        nc.gpsimd.dma_start(out=gb_sb, in_=gb_bc)
        gamma_sb = gb_sb[:, 0]
        beta_sb = gb_sb[:, 1]

        # ---- per-head ot output staging (fp8, all heads resident) ----
        ot_all = singles.tile([P, H, NCH, ND, CH], FP8)

        # ---- PE warmup: dummy matmuls while input DMAs land (HAM clock) ----
        warm = prs.tile([P, CH], F32, tag="rs", name="warm")
        for _ in range(48):
            nc.tensor.matmul(warm[:, 0:P], lhsT=ident, rhs=ident,
                             start=True, stop=True)

        out_r = out.rearrange("(n p) d -> p n d", p=P)

        # tail for one chunk: fc over all heads (PSUM-accumulated) + LN
        def emit_tail(ch):
            for r1 in range(CH // P):
                idx = ch * (CH // P) + r1
                fcp = pot.tile([P, CH], F32, tag="ot", name="fcp")
                for hh in range(H):
                    nc.tensor.matmul(
                        fcp[:, 0:C],
                        lhsT=ot_all[:, hh, ch, :, r1 * P:(r1 + 1) * P],
                        rhs=wfc_sb[:, :, hh, :],
                        start=(hh == 0), stop=(hh == H - 1), perf_mode=DR,
                    )
                u = lnpool.tile([P, C], F32, tag="u", bufs=3)
                # u = fc_psum + 256*(resid + bfc_eff); LN is scale-invariant
                nc.vector.tensor_tensor(out=u, in0=fcp[:, 0:C],
                                        in1=xqf_sb[:, idx], op=OP.add)
                stats = lnpool.tile([P, 6], F32, tag="stats")
                nc.vector.bn_stats(out=stats, in_=u)
                mv = lnpool.tile([P, 2], F32, tag="mv")
                nc.vector.bn_aggr(out=mv, in_=stats)
                sd = lnpool.tile([P, 1], F32, tag="sd")
                nc.scalar.activation(out=sd, in_=mv[:, 1:2], func=AF.Sqrt,
                                     bias=eps_t, scale=1.0)
                rstd = lnpool.tile([P, 1], F32, tag="rstd")
                nc.vector.reciprocal(out=rstd, in_=sd)
                # (u - mu)*gamma on DVE, *rstd on ACT, +beta on GpSimd
                nc.vector.scalar_tensor_tensor(
                    out=u, in0=u, scalar=mv[:, 0:1], in1=gamma_sb,
                    op0=OP.subtract, op1=OP.mult)
                nc.scalar.activation(out=u, in_=u, func=AF.Copy, scale=rstd)
                nc.gpsimd.tensor_tensor(out=u, in0=u, in1=beta_sb, op=OP.add)
                nc.gpsimd.dma_start(out=out_r[:, idx:idx + 1, :], in_=u)

        # ---- head loop ----
        # Projections: qt = 16*(Wq x^T + bq), then G = 2Wk^T @ qt, so the
        # scores matmul is x @ G with a STATIC lhsT (xt8) — no K projection
        # at all.  Head h+1's projection groups are interleaved into head
        # h's chunk streams (one PSUM group per odd tp) and the first scores
        # of head h+1 are prefetched before head h's last AV, so the PE
        # stream is seamless across heads.  Drains alternate ACT/DVE so
        # consecutive PSUM slots free in parallel.
        def make_head_ctx(h):
            qt = hpool.tile([P, ND, RQ], FP8, tag="qt", name="qt")
            gt = hpool.tile([P, ND, RQ], FP8, tag="gt", name="gt")
            v_sb = hpool.tile([P, NT, C], FP8, tag="v", name="v_sb")
            e_tiles = [epool.tile([P, NT, CH], FP8, tag="e", name=f"e{c}")
                       for c in range(NCH)]

            def emit_q(d2, eng):
                qps = pst.tile([P, 1024], F32, tag="st", name="qps")
                for i in range(2):
                    nc.tensor.matmul(
                        qps[:, i * 512:(i + 1) * 512],
                        lhsT=wq_sb[:, :, h, d2 * P:(d2 + 1) * P],
                        rhs=xt_sb[:, :, i * 512:(i + 1) * 512],
                        start=True, stop=True, perf_mode=DR,
                    )
                if eng == "act":
                    nc.scalar.activation(
                        out=qt[:, d2, :], in_=qps, func=AF.Identity,
                        bias=bqk_sb[:, d2, h:h + 1], scale=1.0)
                else:
                    nc.vector.tensor_scalar_add(
                        out=qt[:, d2, :], in0=qps,
                        scalar1=bqk_sb[:, d2, h:h + 1])

            def emit_g(c2, eng):
                gps = pst.tile([P, 1024], F32, tag="st", name="gps")
                for i in range(2):
                    nc.tensor.matmul(
                        gps[:, i * 512:(i + 1) * 512],
                        lhsT=wk_sb[:, :, h, c2 * P:(c2 + 1) * P],
                        rhs=qt[:, :, i * 512:(i + 1) * 512],
                        start=True, stop=True, perf_mode=DR,
                    )
                if eng == "act":
                    nc.scalar.activation(out=gt[:, c2, :], in_=gps,
                                         func=AF.Copy)
                else:
                    nc.vector.tensor_copy(out=gt[:, c2, :], in_=gps)

            def emit_v(q4, eng):
                vps = pst.tile([P, 1024], F32, tag="st", name="vps")
                for i in range(4):
                    t = q4 * 4 + i
                    nc.tensor.matmul(
                        vps[:, i * C:(i + 1) * C],
                        lhsT=xt_sb[:, :, t * P:(t + 1) * P],
                        rhs=wv_sb[:, :, h, :],
                        start=True, stop=True, perf_mode=DR,
                    )
                if eng == "act":
                    nc.scalar.activation(out=v_sb[:, q4 * 4:(q4 + 1) * 4, :],
                                         in_=vps, func=AF.Copy)
                else:
                    nc.vector.tensor_copy(out=v_sb[:, q4 * 4:(q4 + 1) * 4, :],
                                          in_=vps)

            thunks = [
                lambda: emit_q(0, "act"),
                lambda: emit_q(1, "dve"),
                lambda: emit_g(0, "dve"),
                lambda: emit_g(1, "act"),
                lambda: emit_v(0, "act"),
                lambda: emit_v(1, "dve"),
                lambda: emit_v(2, "act"),
                lambda: emit_v(3, "dve"),
            ]
            return dict(qt=qt, gt=gt, v=v_sb, e=e_tiles, thunks=thunks)

        def emit_scores(hc, ch, tp):
            stp = pst.tile([P, 1024], F32, tag="st", name="stp")
            for i in range(2):
                t = tp * 2 + i
                nc.tensor.matmul(
                    stp[:, i * CH:(i + 1) * CH],
                    lhsT=xt_sb[:, :, t * P:(t + 1) * P],
                    rhs=hc["gt"][:, :, ch * CH:(ch + 1) * CH],
                    start=True, stop=True, perf_mode=DR,
                )
            # e = exp(scores/512 - 3), fp8; -3 cancels in softmax
            nc.scalar.activation(out=hc["e"][ch][:, 2 * tp:2 * tp + 2, :],
                                 in_=stp, func=AF.Exp,
                                 scale=float(SCALE_EXP), bias=nb2_t)

        cur = make_head_ctx(0)
        for t_ in cur["thunks"]:
            t_()
        emit_scores(cur, 0, 0)
        for h in range(H):
            nxt = make_head_ctx(h + 1) if h < H - 1 else None
            pend = list(nxt["thunks"]) if nxt else []
            for ch in range(NCH):
                e_sb = cur["e"][ch]
                v_sb = cur["v"]
                otp = [pot.tile([P, CH], F32, tag="ot", name=f"ot{d2}")
                       for d2 in range(ND)]
                rs = prs.tile([P, CH], F32, tag="rs", name="rs")

                for tp in range(NT // 2):
                    if tp < NT // 2 - 1:
                        emit_scores(cur, ch, tp + 1)
                    elif ch == 0:
                        emit_scores(cur, 1, 0)
                    elif nxt is not None:
                        emit_scores(nxt, 0, 0)  # cross-head prefetch
                    nc.tensor.matmul(rs, lhsT=ones2,
                                     rhs=e_sb[:, 2 * tp:2 * tp + 2, :],
                                     start=(tp == 0), stop=(tp == NT // 2 - 1),
                                     perf_mode=DR)
                    for d2 in range(ND):
                        nc.tensor.matmul(
                            otp[d2],
                            lhsT=v_sb[:, 2 * tp:2 * tp + 2, d2 * P:(d2 + 1) * P],
                            rhs=e_sb[:, 2 * tp:2 * tp + 2, :],
                            start=(tp == 0), stop=(tp == NT // 2 - 1),
                            perf_mode=DR,
                        )
                    if tp % 2 == 1 and pend:
                        pend.pop(0)()

                rcp = opool.tile([P, CH], F32, tag="rcp")
                nc.vector.reciprocal_approx_fast(out=rcp, in_=rs)
                for d2 in range(ND):
                    nc.vector.tensor_tensor(
                        out=ot_all[:, h, ch, d2, :], in0=otp[d2], in1=rcp,
                        op=OP.mult)
                # chunk-0 tail overlaps the last head's chunk-1 attention
                if h == H - 1:
                    emit_tail(ch)
            cur = nxt

    nc.finalize()
    return nc


_NC = None


def _get_nc():
    global _NC
    if _NC is None:
        _NC = build_nc()
    return _NC


def make_in_maps(inputs):
    import ml_dtypes
    f8 = ml_dtypes.float8_e4m3
    x = np.asarray(inputs["x"], dtype=np.float32)
    wq = np.asarray(inputs["Wq"], np.float32) * WQK_SCALE
    wk = np.asarray(inputs["Wk"], np.float32) * WKT_SCALE
    wv = np.asarray(inputs["Wv"], np.float32) * WV_SCALE
    wfc = np.asarray(inputs["Wfc"], np.float32)
    bfc_eff = (np.asarray(inputs["bfc"], np.float32).ravel()
               + np.asarray(inputs["bv"], np.float32).ravel() @ wfc)

    def pack_w(w):  # [H, C, C] -> [P, ND, H, C]
        return np.ascontiguousarray(
            w.reshape(H, ND, P, C).transpose(2, 1, 0, 3).astype(f8))

    shared = {
        "wq8": pack_w(wq),
        "wkT8": pack_w(np.ascontiguousarray(wk.transpose(0, 2, 1))),
        "wv8": pack_w(wv),
        "wfc8": pack_w((wfc * WFC_SCALE).reshape(H, C, C)),
        "bqk": np.ascontiguousarray(
            (np.asarray(inputs["bq"], np.float32) * WQK_SCALE)
            .reshape(H, ND, P).transpose(2, 1, 0)),
        "gb": np.ascontiguousarray(np.stack([
            np.asarray(inputs["gamma"], np.float32).ravel(),
            np.asarray(inputs["beta"], np.float32).ravel(),
        ])),
    }
    in_maps = []
    for core in range(8):
        b, r0 = core // 2, (core % 2) * RQ
        m = dict(shared)
        xr = np.roll(x[b], -r0, axis=0)  # query rows first
        m["xt8"] = np.ascontiguousarray(
            xr.T.reshape(ND, P, S).transpose(1, 0, 2).astype(f8))
        m["xqf"] = np.ascontiguousarray(
            ((x[b, r0:r0 + RQ] + bfc_eff[None, :]) * RES_SCALE)
            .reshape(NR, P, C).transpose(1, 0, 2))
        in_maps.append(m)
    return in_maps


def assemble(results):
    out = np.empty((B, S, C), dtype=np.float32)
    for core in range(8):
        b, r0 = core // 2, (core % 2) * RQ
        out[b, r0:r0 + RQ] = results[core]["out"].reshape(RQ, C)
    return out


def kernel(**inputs) -> np.ndarray:
    from concourse.bass_utils import run_bass_kernel_spmd

    nc = _get_nc()
    in_maps = make_in_maps(inputs)
    res = run_bass_kernel_spmd(nc, in_maps, core_ids=list(range(8)))
    return assemble(res.results)
